# revision 42
# baseline (speedup 1.0000x reference)
"""Multi-head self-attention (B=4, N=2048, C=768, H=12, D=64) on 8 TRN2 NeuronCores.

Sharding: (batch, head-group) — core c handles batch c//2, heads (c%2)*6..(c%2)*6+5.
Each core computes its 6 heads' attention plus the partial output projection;
the host sums the two partials per batch and adds the bias terms.

Per-core dataflow (all transpose-free):
  inputs (host-prepped, bf16):
    xt  [896, 2048]  x[b].T padded: rows 0..767 = x.T, row 768 = ones, rest 0
    wq  [896, 1152]  cols [q(384) | k(384) | v(384)] for this core's heads;
                     row 768 = [q bias | k bias | 0]
    wp  [384, 768]   proj_w rows for this core's heads
  phase 1 (ridden as fillers inside the attention loop):
    QT,KT [384, 2048] = wq[:, :768].T @ xt   (bias added on psum drain)
    V_aug [2048, (6, 128)] = xt.T @ wq[:, 768:]  (+ ones blocks)
  attention, per head PAIR (hA even on partitions 0:64, hB odd on 64:128),
  per q-chunk of 512, per k-tile m of 128:
    S^T pair tile [128, 1024] = two ROW-TILED concurrent K=64 matmuls
      (A: rows 0-63 -> psum cols 0:512, B: rows 64-127 -> cols 512:1024)
    exp split by half: A half on the scalar engine (table exp, scale fused),
      B half on the vector engine (deg-3 poly^4) — the two engines stream
      concurrently so exp is never the pipeline pacer
    mm3 per head: psum[128,512] += vv_m^T @ e_half (ones cols give denominator)
    PE queue order: mm2(m), mm3(m-1) — so PE never waits on the exp of m
  drain + normalize per (pair, chunk): reciprocal of the denominator row via
  DRAM round-trip partition-broadcast (heads A/B on separate DMA queues),
  multiply into OUT^T (head A on DVE, head B on Pool)
  proj: partial = OUT^T.T @ wp -> DRAM (fillers, per 128-token tile)
Host: out[b] = part[2b] + part[2b+1] + (qkv_b_v @ proj_w + proj_b)

Engine budget per m-iteration (~1.25us target, PE-bound):
  PE:   mm2 pair (~430ns bus) + mm3 pair (~430) + ~1.5 filler units (~320)
  ACT:  exp A-half (~700) + psum drains (vv casts, proj stage, un B)
  DVE:  exp B-half (~730) + qkt drains + un A + recip + mul A
  Pool: memsets, mul B, head-B normalize DMA chain (queue)
  Sync: input loads, head-A normalize chain, out DMAs
"""

import numpy as np
import ml_dtypes

B, N, C = 4, 2048, 768
H, D = 12, 64
SCALE = D ** -0.5
HL = 6            # heads per core
QK = HL * D       # 384, width of q (= k = v) section per core
KS = 7            # K subtiles (896 = 7*128 rows incl ones/bias row + pad)
KC = KS - 1       # 6 contraction subtiles actually used by matmuls
P = 128
NT = N            # tokens
SC = 512          # q-chunk width / phase-1 chunk width
NCH = NT // SC    # 4 q-chunks
MT = NT // P      # 16 k-tiles

_cache = {}

# DVE polynomial exp: exp(SCALE*x) ~= p(x)^4 with p a deg-3 Horner whose
# coefficients fold in SCALE/4 (minimax on the logit range |SCALE*x|<=2.1,
# max rel err ~2e-3 + bf16 out rounding). Runs on the Vector engine for the
# B-head half of every S tile so the exp stream is split across ACT+DVE.
_EXP_S = SCALE / 4
EXP_C0 = 0.16341808 * _EXP_S ** 3   # s0
EXP_C1 = 0.50925128 * _EXP_S ** 2   # s1
EXP_C2 = 1.00096638 * _EXP_S        # imm2


def _register_exp_op():
    from concourse import dve_ops as D
    from concourse.dve_spec import Spec, Src0, C0, C1, C2, One, sq, lower
    from concourse.dve_uop import DveOpSpec

    for op in D.OPS:
        if op.name == "EXP_POLY_ANT":
            return op
    body = sq(sq(((Src0 * C0 + C1) * Src0 + C2) * Src0 + One))
    spec = Spec(body=body)
    row = D._CUSTOM_DVE_ROW_BASE + len(D.OPS)
    shas = {}
    for ver in ("v3", "v4"):
        tmp = DveOpSpec(name="EXP_POLY_ANT", opcode=row,
                        uops=lower(spec, ver=ver), rd1_en=False)
        shas[ver] = tmp.sha(ver)
    op = D.DveOp("EXP_POLY_ANT", spec, subdim=False, uops_sha=shas)
    D.OPS.append(op)
    D._SUB_OPCODE_FOR_NAME[op.name] = row
    D.CUSTOM_DVE_SPECS[op.name] = spec
    return op


def _build():
    import concourse.bass as bass
    import concourse.mybir as mybir
    import concourse.tile as tile
    from concourse import bacc

    f32 = mybir.dt.float32
    bf16 = mybir.dt.bfloat16
    f8 = mybir.dt.float8e4

    nc = bacc.Bacc(None, target_bir_lowering=False)
    xt_d = nc.declare_dram_parameter("xt", [KS * P, NT], bf16, isOutput=False)
    wq_d = nc.declare_dram_parameter("wq", [KS * P, 3 * QK], bf16, isOutput=False)
    wp_d = nc.declare_dram_parameter("wp", [QK, C], bf16, isOutput=False)
    bias_d = nc.declare_dram_parameter("bias_qk", [P, 2 * QK // P], f32, isOutput=False)
    out_d = nc.declare_dram_parameter("out", [NT, C], f32, isOutput=True)

    xt_r = xt_d.rearrange("(o p) n -> p o n", p=P)
    wq_r = wq_d.rearrange("(o p) n -> p o n", p=P)
    wp_r = wp_d.rearrange("(o p) n -> p o n", p=P)

    exp_op = _register_exp_op()

    with tile.TileContext(nc) as tc:
        with (
            tc.tile_pool(name="persist", bufs=1) as persist,
            tc.tile_pool(name="e_pool", bufs=12) as e_pool,
            tc.tile_pool(name="un_pool", bufs=4) as un_pool,
            tc.tile_pool(name="rec_pool", bufs=4) as rec_pool,
            tc.tile_pool(name="bc_pool", bufs=4) as bc_pool,
            tc.tile_pool(name="stage_pool", bufs=3) as stage_pool,
            tc.tile_pool(name="dr", bufs=4, space="DRAM") as dr_pool,
            tc.tile_pool(name="psS", bufs=2, space="PSUM") as psS,
            tc.tile_pool(name="psO", bufs=1, space="PSUM") as psO,
            tc.tile_pool(name="psF", bufs=2, space="PSUM") as psF,
        ):
            # input SBUF tiles are split per DMA so every consumer's
            # semaphore wait targets exactly the transfers it needs (one
            # merged tile makes the first matmul wait for ALL writes to it)
            xt0 = [persist.tile([P, SC], bf16, name=f"xt0_{o}")
                   for o in range(KC)]                   # chunk 0, per subtile
            xtc = [None] + [persist.tile([P, KC, SC], bf16, name=f"xtc{j}")
                            for j in range(1, NCH)]      # chunks 1-3
            wqk = [persist.tile([P, QK], bf16, name=f"wqk{o}")
                   for o in range(KC)]                   # k section, per subtile
            wqq = persist.tile([P, KC, QK], bf16)        # q section
            wqv = persist.tile([P, KC, QK], bf16)        # v section
            wp = persist.tile([P, QK // P, C], bf16)
            bias_qk = persist.tile([P, 2 * QK // P], f32)
            # per-head Q^T/K^T: head h's 64 dims live at partitions
            # (h%2)*64..+64; the other half is never touched (row-tiled
            # matmuls only read the live half).
            qt = [persist.tile([P, NT], bf16, name=f"qt{h}") for h in range(HL)]
            kt = [persist.tile([P, NT], bf16, name=f"kt{h}") for h in range(HL)]
            # V_aug in fp8e4 (e and v both fp8: mm3 runs in DoubleRow perf
            # mode — two k-tiles contracted per pass at 0.5 cycles/row, ~4x
            # bf16 mm3 throughput; adds ~1e-2 rel err, gate is 2e-2).
            # Layout [P, k-tile-pair, pair-member, head, col].
            vv = persist.tile([P, MT // 2, 2, HL, P], f8)
            outt = [persist.tile([P, NT], bf16, name=f"outt{o}")
                    for o in range(QK // P)]            # normalized out^T

            eng = [nc.sync, nc.gpsimd]   # normalize DMA queues: head A / head B

            def xt_ap(o, lo, hi):
                # view of x^T subtile o, token range [lo, hi) (chunk-aligned)
                j = lo // SC
                if j == 0:
                    return xt0[o][:, lo:hi]
                return xtc[j][:, o, lo - j * SC:hi - j * SC]

            # critical-path loads interleaved across both DGE queues in
            # first-need order: kt needs (wqk[o], xt0[o]) pairs, then wqq,
            # then wqv + later xt chunks. Queue transfers serialize per
            # queue, so issue order = priority. The scalar (ACT) queue
            # carries nothing — it must stay free for the exp stream.
            nc.gpsimd.dma_start(bias_qk[:, :], bias_d[:, :])
            for o in range(KC):
                qa = nc.sync if o % 2 == 0 else nc.gpsimd
                qb = nc.gpsimd if o % 2 == 0 else nc.sync
                qa.dma_start(wqk[o][:, :], wq_r[:, o, QK:2 * QK])
                qb.dma_start(xt0[o][:, :], xt_r[:, o, 0:SC])
            nc.sync.dma_start(wqq[:, :, :], wq_r[:, 0:KC, 0:QK])
            nc.gpsimd.dma_start(wqv[:, :, :], wq_r[:, 0:KC, 2 * QK:3 * QK])
            for j in range(1, NCH):
                nc.sync.dma_start(
                    xtc[j][:, :, :], xt_r[:, 0:KC, j * SC:(j + 1) * SC])

            nc.gpsimd.dma_start(wp[:, :, :], wp_r[:, :, :])
            # V_aug col layout (M=128 so psum writes start at partition 0):
            #   even heads: [v(64) | ones(32) | zeros(32)]  -> den row 64
            #   odd  heads: [zeros(32) | ones(32) | v(64)]  -> den row 32
            # memsets on the gpsimd engine AFTER its input DMAs (~12us of
            # memset on the DVE queue would delay the first poly exp by
            # ~5us; head h's regions are only needed by pair h//2's first
            # mm3, long after the gpsimd queue drains)
            for h in range(HL):
                nc.gpsimd.memset(
                    vv[:, :, :, h, 32:96] if h % 2 else vv[:, :, :, h, 64:96], 1.0)
                nc.gpsimd.memset(
                    vv[:, :, :, h, 0:32] if h % 2 else vv[:, :, :, h, 96:128], 0.0)

            # fillers: independent PE work injected into the exp-wait slots.
            # urgent_sched is a strict per-iteration schedule (one SLOT per
            # run_filler call, each slot = a list of whole unit groups) built
            # in earliest-deadline order so every group is emitted before its
            # consuming matmul; lazy groups smooth into later slots.
            #
            # CORRECTNESS: the tile framework derives dataflow from EMISSION
            # order, so any unit writing a tensor a main-loop matmul reads
            # must be emitted first.
            # GROUP-ATOMICITY: a partially-run psum-accumulation group must
            # finish before a group of another key starts, else the psF
            # 2-buffer rotation hands a third group the bank a partial group
            # is accumulating in and its start=True wipes the partial sums.
            urgent_sched = []
            lazy = []       # qkt chunk groups: no outt dep, pop any time
            lazy_out = []   # proj groups: outt dep, pop only late in a chunk

            def run_filler(m):
                if urgent_sched:
                    for g in urgent_sched.pop(0):
                        run_group(g)
                elif lazy and m % 2 == 0:
                    run_group(lazy.pop(0))
                elif lazy_out and m >= 12:
                    # proj groups read outt written by the PREVIOUS chunk's
                    # normalize mul, which lands ~m==11 of this chunk (the
                    # deferred finish chain carries multi-us completion-
                    # semaphore latency per DMA hop) — an earlier pop would
                    # head-of-line-block the in-order PE queue
                    run_group(lazy_out.pop(0))

            def run_group(units):
                for u in units:
                    u()

            def qkt_chunk_units(mi, j):
                # one 512-col chunk of rows mi*128.. of [Q^T; K^T] (mi<3 -> Q)
                dst = qt if mi < 3 else kt
                ti = mi % 3
                cell = {}

                def unit(o, mi=mi, j=j):
                    if o == 0:
                        cell["ps"] = psF.tile([P, SC], f32, tag="fps", name="ps_f")
                    if mi < 3:
                        lhsT = wqq[:, o, mi * P:(mi + 1) * P]
                    else:
                        lhsT = wqk[o][:, (mi - 3) * P:(mi - 2) * P]
                    nc.tensor.matmul(
                        cell["ps"][:, :SC],
                        lhsT=lhsT,
                        rhs=xt_ap(o, j * SC, (j + 1) * SC),
                        start=(o == 0),
                        stop=(o == KC - 1),
                    )
                    if o == KC - 1:
                        # drain split across ACT/DVE so neither queue eats
                        # the whole ~900ns while the exp stream runs
                        sc = slice(j * SC, (j + 1) * SC)
                        nc.scalar.add(
                            out=dst[2 * ti][0:64, sc], in_=cell["ps"][0:64, :SC],
                            add=bias_qk[0:64, mi:mi + 1])
                        nc.vector.tensor_scalar_add(
                            out=dst[2 * ti + 1][64:P, sc], in0=cell["ps"][64:P, :SC],
                            scalar1=bias_qk[64:P, mi:mi + 1])
                return [lambda o=o: unit(o) for o in range(KC)]

            def v_mtile_units(ti):
                cell = {}

                def unit(o, ti=ti):
                    # subtile KS-1 is exactly zero for V (bias row of the
                    # v-section is zero and the pad rows are zero) - skip it
                    if o == 0:
                        cell["ps"] = psF.tile([P, SC], f32, tag="fps", name="ps_f")
                    nc.tensor.matmul(
                        cell["ps"][:, :QK],
                        lhsT=xt_ap(o, ti * P, (ti + 1) * P),
                        rhs=wqv[:, o, :],
                        start=(o == 0),
                        stop=(o == KC - 1),
                    )
                    if o == KC - 1:
                        psv = cell["ps"][:, :QK].rearrange("p (h d) -> p h d", h=HL)
                        # psum->fp8 casts on ACT (DVE carries the poly exp)
                        nc.scalar.copy(out=vv[:, ti // 2, ti % 2, 0:HL:2, 0:64],
                                       in_=psv[:, 0:HL:2, :])
                        nc.scalar.copy(out=vv[:, ti // 2, ti % 2, 1:HL:2, 64:128],
                                       in_=psv[:, 1:HL:2, :])
                return [lambda o=o: unit(o) for o in range(KC)]

            def proj_units(ti):
                cell = {}

                def unit(pi, o, w0, wn, ti=ti):
                    if pi == 0 and o == 0:
                        cell["stage"] = stage_pool.tile([P, C], f32, name="stage")
                    if o == 0:
                        cell["ps"] = psF.tile([P, SC], f32, tag="fps", name="ps_pj")
                    nc.tensor.matmul(
                        cell["ps"][:, :wn],
                        lhsT=outt[o][:, ti * P:(ti + 1) * P],
                        rhs=wp[:, o, w0:w0 + wn],
                        start=(o == 0),
                        stop=(o == QK // P - 1),
                    )
                    if o == QK // P - 1:
                        nc.scalar.copy(out=cell["stage"][:, w0:w0 + wn],
                                       in_=cell["ps"][:, :wn])
                        if pi == 1:
                            nc.sync.dma_start(out_d[ti * P:(ti + 1) * P, :],
                                              cell["stage"][:, :])
                return [lambda pi=pi, o=o, w0=w0, wn=wn: unit(pi, o, w0, wn)
                        for pi, (w0, wn) in enumerate([(0, 512), (512, 256)])
                        for o in range(QK // P)]

            def normalize_pair(hA, ps_oA, hB, ps_oB, cs, tail=False):
                # phase 0 (emitted NOW): drain both psums on two engines in
                # parallel (the next chunk's mm3 start only waits on these)
                # and launch the denominator-row partition-scatter DMAs
                # ([1,cw] -> [128,cw/128]; a 1-partition DVE reciprocal
                # costs ~4us, the 128-lane one ~0.2us).
                # phase 1/2 (returned as closures, emitted a few iterations
                # into the NEXT chunk): 128-lane reciprocal + DRAM
                # round-trip partition-broadcast, then the normalize muls.
                # Deferring them means their inputs (DMA round trips with
                # ~1.3us completion-semaphore latency) have already landed,
                # so they never head-of-line-block the DVE/Pool queues.
                # Head A rides sync-queue, head B gpsimd.
                cw = SC
                uns, dnps, rcps, bcs = {}, {}, {}, {}
                for idx, (h, ps_o) in enumerate(((hA, ps_oA), (hB, ps_oB))):
                    un = un_pool.tile([P, cw], f32, tag="un", name="un")
                    if idx == 0:
                        nc.vector.tensor_copy(out=un[:, :], in_=ps_o[:, :])
                    else:
                        nc.scalar.copy(out=un[:, :], in_=ps_o[:, :])
                    uns[h] = un
                for idx, h in enumerate((hA, hB)):
                    dlane = 64 if h % 2 == 0 else 32
                    dnp = rec_pool.tile([P, cw // P], f32, name="dnp", tag="dnp")
                    eng[idx].dma_start(dnp[:, :], uns[h][dlane:dlane + 1, 0:cw])
                    dnps[h] = dnp

                def finish1():
                    for idx, h in enumerate((hA, hB)):
                        rcp = rec_pool.tile([P, cw // P], f32, name="rcp", tag="rcp")
                        nc.vector.reciprocal(rcp[:, :], dnps[h][:, :])
                        rcps[h] = rcp
                    for idx, h in enumerate((hA, hB)):
                        de = eng[idx]
                        rd = dr_pool.tile([1, cw], f32, name="rd", tag="rd")
                        de.dma_start(rd[0].rearrange("(p f) -> p f", p=P),
                                     rcps[h][:, :])
                        bc = bc_pool.tile([P, cw], f32, name="bc", tag="bc")
                        de.dma_start(
                            bc[:, :],
                            bass.AP(tensor=rd.tensor, offset=rd.offset,
                                    ap=[[0, P]] + list(rd.ap)),
                        )
                        bcs[h] = bc

                def finish2():
                    # both muls on Pool: it idles otherwise, and a wait at
                    # its queue head blocks nothing (DVE must stay clear
                    # for the poly stream)
                    for idx, h in enumerate((hA, hB)):
                        t, po = h // 2, (h % 2) * 64
                        mul = (nc.vector.tensor_mul if tail
                               else nc.gpsimd.tensor_mul)
                        mul(outt[t][po:po + 64, cs], uns[h][po:po + 64, 0:cw],
                            bcs[h][po:po + 64, :])

                if tail:
                    finish1()
                    finish2()
                    return None
                return finish1, finish2

            pending = [None]      # deferred normalize finish of chunk c-2
            prev_close = [None]   # emits mm3(15) + normalize ph0 of chunk c-1

            def pair_chunk(pr, c):
                # one q-chunk of 512 for head pair (2*pr, 2*pr+1): row-tiled
                # S^T pair per k-tile, exp split A-half on ACT / B-half on
                # DVE (poly), per-head mm3 with the PE-order trick (mm3 of
                # m-1 queued after mm2 of m). The LAST mm3 + normalize
                # drains of a chunk are emitted at m==0 of the NEXT chunk
                # (cross-chunk software pipeline) so the exp stream never
                # drains at a chunk boundary.
                hA, hB = 2 * pr, 2 * pr + 1
                qs = slice(c * SC, (c + 1) * SC)
                ps_oA = psO.tile([P, SC], f32, tag="oa", name="ps_oa")
                ps_oB = psO.tile([P, SC], f32, tag="ob", name="ps_ob")
                es = [None] * (MT // 2)
                for m in range(MT):
                    ps = psS.tile([P, 2 * SC], f32, tag="ps", name="ps_s")
                    ms = slice(m * P, (m + 1) * P)
                    nc.tensor.matmul(
                        ps[:, 0:SC], lhsT=kt[hA][0:64, ms], rhs=qt[hA][0:64, qs],
                        start=True, stop=True,
                    )
                    nc.tensor.matmul(
                        ps[:, SC:2 * SC], lhsT=kt[hB][64:P, ms], rhs=qt[hB][64:P, qs],
                        start=True, stop=True,
                    )
                    # e pair tile [P, pair-member, head-half * SC] in fp8:
                    # member m%2 holds exp of k-tile m, matching vv's pair
                    # layout for the DoubleRow mm3
                    if m % 2 == 0:
                        es[m // 2] = e_pool.tile([P, 2, 2 * SC], f8, name="e")
                    e = es[m // 2]
                    nc.scalar.activation(
                        e[:, m % 2, 0:SC], ps[:, 0:SC],
                        mybir.ActivationFunctionType.Exp,
                        scale=float(SCALE),
                    )
                    nc.vector._custom_dve(exp_op, out=e[:, m % 2, SC:2 * SC],
                                          in0=ps[:, SC:2 * SC],
                                          s0=EXP_C0, s1=EXP_C1, imm2=EXP_C2)
                    if m == 0 and prev_close[0] is not None:
                        prev_close[0](False)
                    if pending[0] is not None:
                        # recips at m==3: their dnp inputs (issued at m==0)
                        # have landed, so no DVE head-of-line wait. During
                        # pair 2 flush the muls right after at m==5: they
                        # camp at the idle Pool queue head until the bc
                        # semaphore (~m==8) and finish by ~m==10, so the
                        # m>=12 proj pops never stall
                        f1, f2 = (3, 5) if pr == 2 else (3, 8)
                        if m == f1:
                            pending[0][0]()
                        elif m == f2:
                            pending[0][1]()
                            pending[0] = None
                    run_filler(m)
                    # consume e pairs two iterations late: mm3(mp) sits on
                    # the in-order PE queue after mm2(2mp+3), so PE never
                    # waits for the exp stream
                    if m % 2 == 1 and m >= 3:
                        mm3(hA, hB, (m - 3) // 2, ps_oA, ps_oB, es[(m - 3) // 2])

                def close(tail):
                    mm3(hA, hB, MT // 2 - 1, ps_oA, ps_oB, es[MT // 2 - 1])
                    pending[0] = normalize_pair(hA, ps_oA, hB, ps_oB, qs,
                                                tail=tail)
                prev_close[0] = close

            def mm3(hA, hB, mp, ps_oA, ps_oB, e):
                # fp8 DoubleRow: contracts k-tile pair (2mp, 2mp+1) in one
                # pass (0.5 cycles/row). lhsT [128, 2, cols], rhs [128, 2,
                # SC] — dim1 is the pair member. Even heads: [v(64)|ones(32)]
                # is all that matters - 96 stationary cols shave the
                # LDWEIGHTS.
                dr = mybir.MatmulPerfMode.DoubleRow
                nc.tensor.matmul(
                    ps_oA[0:96, :], lhsT=vv[:, mp, :, hA, 0:96],
                    rhs=e[:, :, 0:SC], perf_mode=dr,
                    start=(mp == 0), stop=(mp == MT // 2 - 1),
                )
                nc.tensor.matmul(
                    ps_oB[:, :], lhsT=vv[:, mp, :, hB, :],
                    rhs=e[:, :, SC:2 * SC], perf_mode=dr,
                    start=(mp == 0), stop=(mp == MT // 2 - 1),
                )

            # pre-work: K^T chunk 0, Q^T chunk 0 of pair 0, and V tile 0 —
            # just enough to start the exp stream. Everything else rides the
            # filler slots; urgent_sched is the hand-scheduled chunk-0 plan
            # (v_m needed by mm3(m) emitted in iteration m+1; kt-j1/2/3 by
            # mm2(4)/(8)/(12); qt(0,1) by pair-0-chunk-1's first mm2).
            run_group(qkt_chunk_units(3, 0))
            run_group(qkt_chunk_units(0, 0))
            run_group(v_mtile_units(0))
            kj = [qkt_chunk_units(3, j) for j in range(1, NCH)]
            vm = [v_mtile_units(ti) for ti in range(1, MT)]
            # qt chunk 1 rides slot 11 (double-popped with v12): computed at
            # the last slot it would finish just-in-time and stall the
            # pc0->pc1 transition ~4us waiting on its psum drains
            urgent_sched.extend([
                [vm[0]], [vm[1], kj[0]], [vm[2]], [vm[3]],
                [vm[4], kj[1]], [vm[5]], [vm[6]], [vm[7]],
                [vm[8]], [vm[9], kj[2]], [vm[10]],
                [vm[11], qkt_chunk_units(0, 1)],
                [vm[12]], [vm[13]], [vm[14]], [],
            ])
            for j in range(2, NCH):
                lazy.append(qkt_chunk_units(0, j))
            for mi in (1, 4):
                for j in range(NCH):
                    lazy.append(qkt_chunk_units(mi, j))

            for pr in range(3):
                if pr == 1:
                    for mi in (2, 5):
                        for j in range(NCH):
                            lazy.append(qkt_chunk_units(mi, j))
                for c in range(NCH):
                    # proj for chunk c-1 becomes available only now: its
                    # outt dep (the normalize muls) needs a full chunk of
                    # latency headroom, else a proj matmul waiting on outt
                    # head-of-line-blocks the in-order PE queue.
                    if pr == 2 and c > 0:
                        for ti in range(4 * (c - 1), 4 * c):
                            lazy_out.append(proj_units(ti))
                    pair_chunk(pr, c)
            # close the final chunk inline (no next chunk to defer into)
            prev_close[0](True)
            while urgent_sched or lazy or lazy_out:
                if urgent_sched:
                    for g in urgent_sched.pop(0):
                        run_group(g)
                elif lazy:
                    run_group(lazy.pop(0))
                else:
                    run_group(lazy_out.pop(0))
            # final proj tiles: front-load the o=0/o=1 partial matmuls for
            # tiles 12-14 into held psum banks (psF/psO/psS are all free at
            # the tail) so they execute during the otherwise PE-idle tail
            # normalize chain; only the o=2 closers wait for the tail muls.
            # Groups are interleaved across DISTINCT banks, so accumulation
            # stays correct; skip the contiguity check.
            fronts = []
            pools = {12: (psF, "fps", "fps"), 13: (psO, "oa", "ob"),
                     14: (psS, "ps", "ps")}
            for ti in (12, 13, 14):
                pool, tg0, tg1 = pools[ti]
                for pi, (w0, wn) in enumerate([(0, 512), (512, 256)]):
                    ps = pool.tile([P, SC], f32, tag=(tg0 if pi == 0 else tg1),
                                   name="ps_tl")
                    for o in (0, 1):
                        nc.tensor.matmul(
                            ps[:, :wn],
                            lhsT=outt[o][:, ti * P:(ti + 1) * P],
                            rhs=wp[:, o, w0:w0 + wn],
                            start=(o == 0), stop=False,
                            skip_group_check=True,
                        )
                    fronts.append((ti, pi, w0, wn, ps))
            stages = {}
            for ti, pi, w0, wn, ps in fronts:
                if pi == 0:
                    stages[ti] = stage_pool.tile([P, C], f32, name="stage")
                nc.tensor.matmul(
                    ps[:, :wn],
                    lhsT=outt[2][:, ti * P:(ti + 1) * P],
                    rhs=wp[:, 2, w0:w0 + wn],
                    start=False, stop=True,
                    skip_group_check=True,
                )
                nc.scalar.copy(out=stages[ti][:, w0:w0 + wn], in_=ps[:, :wn])
                if pi == 1:
                    nc.sync.dma_start(out_d[ti * P:(ti + 1) * P, :],
                                      stages[ti][:, :])
            run_group(proj_units(15))

    nc.compile()
    return nc


def _prep_inputs(x, qkv_w, qkv_b):
    bf = ml_dtypes.bfloat16
    in_maps = []
    for c in range(8):
        b, hs = c // 2, (c % 2) * HL
        xt = np.zeros((KS * P, NT), dtype=bf)
        xt[0:C, :] = x[b].T.astype(bf)
        xt[C, :] = 1.0
        wq = np.zeros((KS * P, 3 * QK), dtype=bf)
        for s in range(3):  # q, k, v sections
            cols = qkv_w[:, s * C + hs * D: s * C + (hs + HL) * D]
            wq[0:C, s * QK:(s + 1) * QK] = cols.astype(bf)
        wq[C, 0:QK] = qkv_b[hs * D:(hs + HL) * D].astype(bf)
        wq[C, QK:2 * QK] = qkv_b[C + hs * D: C + (hs + HL) * D].astype(bf)
        qk_bias = np.concatenate([
            qkv_b[hs * D:(hs + HL) * D], qkv_b[C + hs * D: C + (hs + HL) * D]
        ]).astype(np.float32)
        in_maps.append({"xt": xt, "wq": wq,
                        "bias_qk": np.ascontiguousarray(qk_bias.reshape(6, P).T)})
    return in_maps


def kernel(x, qkv_w, qkv_b, proj_w, proj_b):
    from concourse.bass_utils import run_bass_kernel_spmd

    x = np.asarray(x, dtype=np.float32)
    qkv_w = np.asarray(qkv_w, dtype=np.float32)
    qkv_b = np.asarray(qkv_b, dtype=np.float32)
    proj_w = np.asarray(proj_w, dtype=np.float32)
    proj_b = np.asarray(proj_b, dtype=np.float32)

    if "nc" not in _cache:
        _cache["nc"] = _build()
    nc = _cache["nc"]

    bf = ml_dtypes.bfloat16
    in_maps = _prep_inputs(x, qkv_w, qkv_b)
    for c in range(8):
        hs = (c % 2) * HL
        in_maps[c]["wp"] = proj_w[hs * D:(hs + HL) * D, :].astype(bf)

    res = run_bass_kernel_spmd(nc, in_maps, core_ids=list(range(8)))
    parts = [res.results[c]["out"].astype(np.float32) for c in range(8)]

    # v-bias contribution (exact, f32) + proj bias, added once per batch
    const_row = qkv_b[2 * C:] @ proj_w + proj_b
    out = np.empty((B, N, C), dtype=np.float32)
    for b in range(B):
        out[b] = parts[2 * b] + parts[2 * b + 1] + const_row
    return out


# revision 43
# speedup vs baseline: 1.0109x; 1.0109x over previous
"""Multi-head self-attention (B=4, N=2048, C=768, H=12, D=64) on 8 TRN2 NeuronCores.

Sharding: (batch, head-group) — core c handles batch c//2, heads (c%2)*6..(c%2)*6+5.
Each core computes its 6 heads' attention plus the partial output projection;
the host sums the two partials per batch and adds the bias terms.

Per-core dataflow (all transpose-free):
  inputs (host-prepped, bf16):
    xt  [896, 2048]  x[b].T padded: rows 0..767 = x.T, row 768 = ones, rest 0
    wq  [896, 1152]  cols [q(384) | k(384) | v(384)] for this core's heads;
                     row 768 = [q bias | k bias | 0]
    wp  [384, 768]   proj_w rows for this core's heads
  phase 1 (ridden as fillers inside the attention loop):
    QT,KT [384, 2048] = wq[:, :768].T @ xt   (bias added on psum drain)
    V_aug [2048, (6, 128)] = xt.T @ wq[:, 768:]  (+ ones blocks)
  attention, per head PAIR (hA even on partitions 0:64, hB odd on 64:128),
  per q-chunk of 512, per k-tile m of 128:
    S^T pair tile [128, 1024] = two ROW-TILED concurrent K=64 matmuls
      (A: rows 0-63 -> psum cols 0:512, B: rows 64-127 -> cols 512:1024)
    exp split by half: A half on the scalar engine (table exp, scale fused),
      B half on the vector engine (deg-3 poly^4) — the two engines stream
      concurrently so exp is never the pipeline pacer
    mm3 per head: psum[128,512] += vv_m^T @ e_half (ones cols give denominator)
    PE queue order: mm2(m), mm3(m-1) — so PE never waits on the exp of m
  drain + normalize per (pair, chunk): reciprocal of the denominator row via
  DRAM round-trip partition-broadcast (heads A/B on separate DMA queues),
  multiply into OUT^T (head A on DVE, head B on Pool)
  proj: partial = OUT^T.T @ wp -> DRAM (fillers, per 128-token tile)
Host: out[b] = part[2b] + part[2b+1] + (qkv_b_v @ proj_w + proj_b)

Engine budget per m-iteration (~1.25us target, PE-bound):
  PE:   mm2 pair (~430ns bus) + mm3 pair (~430) + ~1.5 filler units (~320)
  ACT:  exp A-half (~700) + psum drains (vv casts, proj stage, un B)
  DVE:  exp B-half (~730) + qkt drains + un A + recip + mul A
  Pool: memsets, mul B, head-B normalize DMA chain (queue)
  Sync: input loads, head-A normalize chain, out DMAs
"""

import numpy as np
import ml_dtypes

B, N, C = 4, 2048, 768
H, D = 12, 64
SCALE = D ** -0.5
HL = 6            # heads per core
QK = HL * D       # 384, width of q (= k = v) section per core
KS = 7            # K subtiles (896 = 7*128 rows incl ones/bias row + pad)
KC = KS - 1       # 6 contraction subtiles actually used by matmuls
P = 128
NT = N            # tokens
SC = 512          # q-chunk width / phase-1 chunk width
NCH = NT // SC    # 4 q-chunks
MT = NT // P      # 16 k-tiles

_cache = {}

# DVE polynomial exp: exp(SCALE*x) ~= p(x)^4 with p a deg-3 Horner whose
# coefficients fold in SCALE/4 (minimax on the logit range |SCALE*x|<=2.1,
# max rel err ~2e-3 + bf16 out rounding). Runs on the Vector engine for the
# B-head half of every S tile so the exp stream is split across ACT+DVE.
_EXP_S = SCALE / 4
EXP_C0 = 0.16341808 * _EXP_S ** 3   # s0
EXP_C1 = 0.50925128 * _EXP_S ** 2   # s1
EXP_C2 = 1.00096638 * _EXP_S        # imm2


def _register_exp_op():
    from concourse import dve_ops as D
    from concourse.dve_spec import Spec, Src0, C0, C1, C2, One, sq, lower
    from concourse.dve_uop import DveOpSpec

    for op in D.OPS:
        if op.name == "EXP_POLY_ANT":
            return op
    body = sq(sq(((Src0 * C0 + C1) * Src0 + C2) * Src0 + One))
    spec = Spec(body=body)
    row = D._CUSTOM_DVE_ROW_BASE + len(D.OPS)
    shas = {}
    for ver in ("v3", "v4"):
        tmp = DveOpSpec(name="EXP_POLY_ANT", opcode=row,
                        uops=lower(spec, ver=ver), rd1_en=False)
        shas[ver] = tmp.sha(ver)
    op = D.DveOp("EXP_POLY_ANT", spec, subdim=False, uops_sha=shas)
    D.OPS.append(op)
    D._SUB_OPCODE_FOR_NAME[op.name] = row
    D.CUSTOM_DVE_SPECS[op.name] = spec
    return op


def _build():
    import concourse.bass as bass
    import concourse.mybir as mybir
    import concourse.tile as tile
    from concourse import bacc

    f32 = mybir.dt.float32
    bf16 = mybir.dt.bfloat16
    f8 = mybir.dt.float8e4

    nc = bacc.Bacc(None, target_bir_lowering=False)
    xt_d = nc.declare_dram_parameter("xt", [KS * P, NT], bf16, isOutput=False)
    wq_d = nc.declare_dram_parameter("wq", [KS * P, 3 * QK], bf16, isOutput=False)
    wp_d = nc.declare_dram_parameter("wp", [QK, C], bf16, isOutput=False)
    bias_d = nc.declare_dram_parameter("bias_qk", [P, 2 * QK // P], f32, isOutput=False)
    out_d = nc.declare_dram_parameter("out", [NT, C], f32, isOutput=True)

    xt_r = xt_d.rearrange("(o p) n -> p o n", p=P)
    wq_r = wq_d.rearrange("(o p) n -> p o n", p=P)
    wp_r = wp_d.rearrange("(o p) n -> p o n", p=P)

    exp_op = _register_exp_op()

    with tile.TileContext(nc) as tc:
        with (
            tc.tile_pool(name="persist", bufs=1) as persist,
            tc.tile_pool(name="e_pool", bufs=12) as e_pool,
            tc.tile_pool(name="un_pool", bufs=4) as un_pool,
            tc.tile_pool(name="rec_pool", bufs=4) as rec_pool,
            tc.tile_pool(name="bc_pool", bufs=4) as bc_pool,
            tc.tile_pool(name="stage_pool", bufs=3) as stage_pool,
            tc.tile_pool(name="dr", bufs=4, space="DRAM") as dr_pool,
            tc.tile_pool(name="psS", bufs=2, space="PSUM") as psS,
            tc.tile_pool(name="psO", bufs=1, space="PSUM") as psO,
            tc.tile_pool(name="psF", bufs=2, space="PSUM") as psF,
        ):
            # input SBUF tiles are split per DMA so every consumer's
            # semaphore wait targets exactly the transfers it needs (one
            # merged tile makes the first matmul wait for ALL writes to it)
            xt0 = [persist.tile([P, SC], bf16, name=f"xt0_{o}")
                   for o in range(KC)]                   # chunk 0, per subtile
            xtc = [None] + [persist.tile([P, KC, SC], bf16, name=f"xtc{j}")
                            for j in range(1, NCH)]      # chunks 1-3
            wqk = [persist.tile([P, QK], bf16, name=f"wqk{o}")
                   for o in range(KC)]                   # k section, per subtile
            wqq = persist.tile([P, KC, QK], bf16)        # q section
            wqv = persist.tile([P, KC, QK], bf16)        # v section
            wp = persist.tile([P, QK // P, C], bf16)
            bias_qk = persist.tile([P, 2 * QK // P], f32)
            # per-head Q^T/K^T: head h's 64 dims live at partitions
            # (h%2)*64..+64; the other half is never touched (row-tiled
            # matmuls only read the live half).
            qt = [persist.tile([P, NT], bf16, name=f"qt{h}") for h in range(HL)]
            kt = [persist.tile([P, NT], bf16, name=f"kt{h}") for h in range(HL)]
            # V_aug in fp8e4 (e and v both fp8: mm3 runs in DoubleRow perf
            # mode — two k-tiles contracted per pass at 0.5 cycles/row, ~4x
            # bf16 mm3 throughput; adds ~1e-2 rel err, gate is 2e-2).
            # Layout [P, k-tile-pair, pair-member, head, col].
            vv = persist.tile([P, MT // 2, 2, HL, P], f8)
            outt = [persist.tile([P, NT], bf16, name=f"outt{o}")
                    for o in range(QK // P)]            # normalized out^T

            eng = [nc.sync, nc.gpsimd]   # normalize DMA queues: head A / head B

            def xt_ap(o, lo, hi):
                # view of x^T subtile o, token range [lo, hi) (chunk-aligned)
                j = lo // SC
                if j == 0:
                    return xt0[o][:, lo:hi]
                return xtc[j][:, o, lo - j * SC:hi - j * SC]

            # critical-path loads interleaved across both DGE queues in
            # first-need order: kt needs (wqk[o], xt0[o]) pairs, then wqq,
            # then wqv + later xt chunks. Queue transfers serialize per
            # queue, so issue order = priority. The scalar (ACT) queue
            # carries nothing — it must stay free for the exp stream.
            nc.gpsimd.dma_start(bias_qk[:, :], bias_d[:, :])
            for o in range(KC):
                qa = nc.sync if o % 2 == 0 else nc.gpsimd
                qb = nc.gpsimd if o % 2 == 0 else nc.sync
                qa.dma_start(wqk[o][:, :], wq_r[:, o, QK:2 * QK])
                qb.dma_start(xt0[o][:, :], xt_r[:, o, 0:SC])
            nc.sync.dma_start(wqq[:, :, :], wq_r[:, 0:KC, 0:QK])
            nc.gpsimd.dma_start(wqv[:, :, :], wq_r[:, 0:KC, 2 * QK:3 * QK])
            for j in range(1, NCH):
                nc.sync.dma_start(
                    xtc[j][:, :, :], xt_r[:, 0:KC, j * SC:(j + 1) * SC])

            nc.gpsimd.dma_start(wp[:, :, :], wp_r[:, :, :])
            # V_aug col layout (M=128 so psum writes start at partition 0):
            #   even heads: [v(64) | ones(32) | zeros(32)]  -> den row 64
            #   odd  heads: [zeros(32) | ones(32) | v(64)]  -> den row 32
            # memsets on the gpsimd engine AFTER its input DMAs (~12us of
            # memset on the DVE queue would delay the first poly exp by
            # ~5us; head h's regions are only needed by pair h//2's first
            # mm3, long after the gpsimd queue drains)
            for h in range(HL):
                nc.gpsimd.memset(
                    vv[:, :, :, h, 32:96] if h % 2 else vv[:, :, :, h, 64:96], 1.0)
                nc.gpsimd.memset(
                    vv[:, :, :, h, 0:32] if h % 2 else vv[:, :, :, h, 96:128], 0.0)

            # fillers: independent PE work injected into the exp-wait slots.
            # urgent_sched is a strict per-iteration schedule (one SLOT per
            # run_filler call, each slot = a list of whole unit groups) built
            # in earliest-deadline order so every group is emitted before its
            # consuming matmul; lazy groups smooth into later slots.
            #
            # CORRECTNESS: the tile framework derives dataflow from EMISSION
            # order, so any unit writing a tensor a main-loop matmul reads
            # must be emitted first.
            # GROUP-ATOMICITY: a partially-run psum-accumulation group must
            # finish before a group of another key starts, else the psF
            # 2-buffer rotation hands a third group the bank a partial group
            # is accumulating in and its start=True wipes the partial sums.
            urgent_sched = []
            lazy = []       # qkt chunk groups: no outt dep, pop any time
            lazy_out = []   # proj groups: outt dep, pop only late in a chunk

            def run_filler(m):
                if urgent_sched:
                    for g in urgent_sched.pop(0):
                        run_group(g)
                elif lazy and m % 2 == 0:
                    run_group(lazy.pop(0))
                elif lazy_out and m >= 12:
                    # proj groups read outt written by the PREVIOUS chunk's
                    # normalize mul, which lands ~m==11 of this chunk (the
                    # deferred finish chain carries multi-us completion-
                    # semaphore latency per DMA hop) — an earlier pop would
                    # head-of-line-block the in-order PE queue
                    run_group(lazy_out.pop(0))

            def run_group(units):
                for u in units:
                    u()

            def qkt_chunk_units(mi, j):
                # one 512-col chunk of rows mi*128.. of [Q^T; K^T] (mi<3 -> Q)
                dst = qt if mi < 3 else kt
                ti = mi % 3
                cell = {}

                def unit(o, mi=mi, j=j):
                    if o == 0:
                        cell["ps"] = psF.tile([P, SC], f32, tag="fps", name="ps_f")
                    if mi < 3:
                        lhsT = wqq[:, o, mi * P:(mi + 1) * P]
                    else:
                        lhsT = wqk[o][:, (mi - 3) * P:(mi - 2) * P]
                    nc.tensor.matmul(
                        cell["ps"][:, :SC],
                        lhsT=lhsT,
                        rhs=xt_ap(o, j * SC, (j + 1) * SC),
                        start=(o == 0),
                        stop=(o == KC - 1),
                    )
                    if o == KC - 1:
                        # drain split across ACT/DVE so neither queue eats
                        # the whole ~900ns while the exp stream runs
                        sc = slice(j * SC, (j + 1) * SC)
                        nc.scalar.add(
                            out=dst[2 * ti][0:64, sc], in_=cell["ps"][0:64, :SC],
                            add=bias_qk[0:64, mi:mi + 1])
                        nc.vector.tensor_scalar_add(
                            out=dst[2 * ti + 1][64:P, sc], in0=cell["ps"][64:P, :SC],
                            scalar1=bias_qk[64:P, mi:mi + 1])
                return [lambda o=o: unit(o) for o in range(KC)]

            def v_mtile_units(ti):
                cell = {}

                def unit(o, ti=ti):
                    # subtile KS-1 is exactly zero for V (bias row of the
                    # v-section is zero and the pad rows are zero) - skip it
                    if o == 0:
                        cell["ps"] = psF.tile([P, SC], f32, tag="fps", name="ps_f")
                    nc.tensor.matmul(
                        cell["ps"][:, :QK],
                        lhsT=xt_ap(o, ti * P, (ti + 1) * P),
                        rhs=wqv[:, o, :],
                        start=(o == 0),
                        stop=(o == KC - 1),
                    )
                    if o == KC - 1:
                        psv = cell["ps"][:, :QK].rearrange("p (h d) -> p h d", h=HL)
                        # psum->fp8 casts on ACT (DVE carries the poly exp)
                        nc.scalar.copy(out=vv[:, ti // 2, ti % 2, 0:HL:2, 0:64],
                                       in_=psv[:, 0:HL:2, :])
                        nc.scalar.copy(out=vv[:, ti // 2, ti % 2, 1:HL:2, 64:128],
                                       in_=psv[:, 1:HL:2, :])
                return [lambda o=o: unit(o) for o in range(KC)]

            def proj_units(ti):
                cell = {}

                def unit(pi, o, w0, wn, ti=ti):
                    if pi == 0 and o == 0:
                        cell["stage"] = stage_pool.tile([P, C], f32, name="stage")
                    if o == 0:
                        cell["ps"] = psF.tile([P, SC], f32, tag="fps", name="ps_pj")
                    nc.tensor.matmul(
                        cell["ps"][:, :wn],
                        lhsT=outt[o][:, ti * P:(ti + 1) * P],
                        rhs=wp[:, o, w0:w0 + wn],
                        start=(o == 0),
                        stop=(o == QK // P - 1),
                    )
                    if o == QK // P - 1:
                        nc.scalar.copy(out=cell["stage"][:, w0:w0 + wn],
                                       in_=cell["ps"][:, :wn])
                        if pi == 1:
                            nc.sync.dma_start(out_d[ti * P:(ti + 1) * P, :],
                                              cell["stage"][:, :])
                return [lambda pi=pi, o=o, w0=w0, wn=wn: unit(pi, o, w0, wn)
                        for pi, (w0, wn) in enumerate([(0, 512), (512, 256)])
                        for o in range(QK // P)]

            def normalize_pair(hA, ps_oA, hB, ps_oB, cs, tail=False):
                # phase 0 (emitted NOW): drain both psums on two engines in
                # parallel (the next chunk's mm3 start only waits on these)
                # and launch the denominator-row partition-scatter DMAs
                # ([1,cw] -> [128,cw/128]; a 1-partition DVE reciprocal
                # costs ~4us, the 128-lane one ~0.2us).
                # phase 1/2 (returned as closures, emitted a few iterations
                # into the NEXT chunk): 128-lane reciprocal + DRAM
                # round-trip partition-broadcast, then the normalize muls.
                # Deferring them means their inputs (DMA round trips with
                # ~1.3us completion-semaphore latency) have already landed,
                # so they never head-of-line-block the DVE/Pool queues.
                # Head A rides sync-queue, head B gpsimd.
                cw = SC
                uns, dnps, rcps, bcs = {}, {}, {}, {}
                for idx, (h, ps_o) in enumerate(((hA, ps_oA), (hB, ps_oB))):
                    un = un_pool.tile([P, cw], f32, tag="un", name="un")
                    if idx == 0:
                        nc.vector.tensor_copy(out=un[:, :], in_=ps_o[:, :])
                    else:
                        nc.scalar.copy(out=un[:, :], in_=ps_o[:, :])
                    uns[h] = un
                for idx, h in enumerate((hA, hB)):
                    dlane = 64 if h % 2 == 0 else 32
                    dnp = rec_pool.tile([P, cw // P], f32, name="dnp", tag="dnp")
                    eng[idx].dma_start(dnp[:, :], uns[h][dlane:dlane + 1, 0:cw])
                    dnps[h] = dnp

                def finish1():
                    for idx, h in enumerate((hA, hB)):
                        rcp = rec_pool.tile([P, cw // P], f32, name="rcp", tag="rcp")
                        nc.vector.reciprocal(rcp[:, :], dnps[h][:, :])
                        rcps[h] = rcp
                    for idx, h in enumerate((hA, hB)):
                        de = eng[idx]
                        rd = dr_pool.tile([1, cw], f32, name="rd", tag="rd")
                        de.dma_start(rd[0].rearrange("(p f) -> p f", p=P),
                                     rcps[h][:, :])
                        bc = bc_pool.tile([P, cw], f32, name="bc", tag="bc")
                        de.dma_start(
                            bc[:, :],
                            bass.AP(tensor=rd.tensor, offset=rd.offset,
                                    ap=[[0, P]] + list(rd.ap)),
                        )
                        bcs[h] = bc

                def finish2():
                    # both muls on Pool: it idles otherwise, and a wait at
                    # its queue head blocks nothing (DVE must stay clear
                    # for the poly stream)
                    for idx, h in enumerate((hA, hB)):
                        t, po = h // 2, (h % 2) * 64
                        mul = (nc.vector.tensor_mul if tail
                               else nc.gpsimd.tensor_mul)
                        mul(outt[t][po:po + 64, cs], uns[h][po:po + 64, 0:cw],
                            bcs[h][po:po + 64, :])

                if tail:
                    finish1()
                    finish2()
                    return None
                return finish1, finish2

            pending = [None]      # deferred normalize finish of chunk c-2
            prev_close = [None]   # emits mm3(15) + normalize ph0 of chunk c-1

            def pair_chunk(pr, c):
                # one q-chunk of 512 for head pair (2*pr, 2*pr+1): row-tiled
                # S^T pair per k-tile, exp split A-half on ACT / B-half on
                # DVE (poly), per-head mm3 with the PE-order trick (mm3 of
                # m-1 queued after mm2 of m). The LAST mm3 + normalize
                # drains of a chunk are emitted at m==0 of the NEXT chunk
                # (cross-chunk software pipeline) so the exp stream never
                # drains at a chunk boundary.
                hA, hB = 2 * pr, 2 * pr + 1
                qs = slice(c * SC, (c + 1) * SC)
                ps_oA = psO.tile([P, SC], f32, tag="oa", name="ps_oa")
                ps_oB = psO.tile([P, SC], f32, tag="ob", name="ps_ob")
                es = [None] * (MT // 2)
                for m in range(MT):
                    ps = psS.tile([P, 2 * SC], f32, tag="ps", name="ps_s")
                    ms = slice(m * P, (m + 1) * P)
                    nc.tensor.matmul(
                        ps[:, 0:SC], lhsT=kt[hA][0:64, ms], rhs=qt[hA][0:64, qs],
                        start=True, stop=True,
                    )
                    nc.tensor.matmul(
                        ps[:, SC:2 * SC], lhsT=kt[hB][64:P, ms], rhs=qt[hB][64:P, qs],
                        start=True, stop=True,
                    )
                    # e pair tile [P, pair-member, head-half * SC] in fp8:
                    # member m%2 holds exp of k-tile m, matching vv's pair
                    # layout for the DoubleRow mm3
                    if m % 2 == 0:
                        es[m // 2] = e_pool.tile([P, 2, 2 * SC], f8, name="e")
                    e = es[m // 2]
                    nc.scalar.activation(
                        e[:, m % 2, 0:SC], ps[:, 0:SC],
                        mybir.ActivationFunctionType.Exp,
                        scale=float(SCALE),
                    )
                    nc.vector._custom_dve(exp_op, out=e[:, m % 2, SC:2 * SC],
                                          in0=ps[:, SC:2 * SC],
                                          s0=EXP_C0, s1=EXP_C1, imm2=EXP_C2)
                    if m == 0 and prev_close[0] is not None:
                        prev_close[0](False)
                    if pending[0] is not None:
                        # recips at m==3: their dnp inputs (issued at m==0)
                        # have landed, so no DVE head-of-line wait. During
                        # pair 2 flush the muls right after at m==5: they
                        # camp at the idle Pool queue head until the bc
                        # semaphore (~m==8) and finish by ~m==10, so the
                        # m>=12 proj pops never stall
                        f1, f2 = (3, 5) if pr == 2 else (3, 8)
                        if m == f1:
                            pending[0][0]()
                        elif m == f2:
                            pending[0][1]()
                            pending[0] = None
                    run_filler(m)
                    # consume e pairs two iterations late: mm3(mp) sits on
                    # the in-order PE queue after mm2(2mp+3), so PE never
                    # waits for the exp stream
                    if m % 2 == 1 and m >= 3:
                        mm3(hA, hB, (m - 3) // 2, ps_oA, ps_oB, es[(m - 3) // 2])

                def close(tail):
                    mm3(hA, hB, MT // 2 - 1, ps_oA, ps_oB, es[MT // 2 - 1])
                    pending[0] = normalize_pair(hA, ps_oA, hB, ps_oB, qs,
                                                tail=tail)
                prev_close[0] = close

            def mm3(hA, hB, mp, ps_oA, ps_oB, e):
                # fp8 DoubleRow: contracts k-tile pair (2mp, 2mp+1) in one
                # pass (0.5 cycles/row). lhsT [128, 2, cols], rhs [128, 2,
                # SC] — dim1 is the pair member. Even heads: [v(64)|ones(32)]
                # is all that matters - 96 stationary cols shave the
                # LDWEIGHTS.
                dr = mybir.MatmulPerfMode.DoubleRow
                nc.tensor.matmul(
                    ps_oA[0:96, :], lhsT=vv[:, mp, :, hA, 0:96],
                    rhs=e[:, :, 0:SC], perf_mode=dr,
                    start=(mp == 0), stop=(mp == MT // 2 - 1),
                )
                nc.tensor.matmul(
                    ps_oB[:, :], lhsT=vv[:, mp, :, hB, :],
                    rhs=e[:, :, SC:2 * SC], perf_mode=dr,
                    start=(mp == 0), stop=(mp == MT // 2 - 1),
                )

            # pre-work: K^T chunk 0, Q^T chunk 0 of pair 0, and V tile 0 —
            # just enough to start the exp stream. Everything else rides the
            # filler slots; urgent_sched is the hand-scheduled chunk-0 plan
            # (v_m needed by mm3(m) emitted in iteration m+1; kt-j1/2/3 by
            # mm2(4)/(8)/(12); qt(0,1) by pair-0-chunk-1's first mm2).
            run_group(qkt_chunk_units(3, 0))
            run_group(qkt_chunk_units(0, 0))
            run_group(v_mtile_units(0))
            kj = [qkt_chunk_units(3, j) for j in range(1, NCH)]
            vm = [v_mtile_units(ti) for ti in range(1, MT)]
            # qt chunk 1 rides slot 11 (double-popped with v12): computed at
            # the last slot it would finish just-in-time and stall the
            # pc0->pc1 transition ~4us waiting on its psum drains
            urgent_sched.extend([
                [vm[0]], [vm[1], kj[0]], [vm[2]], [vm[3]],
                [vm[4], kj[1]], [vm[5]], [vm[6]], [vm[7]],
                [vm[8]], [vm[9], kj[2]], [vm[10]],
                [vm[11], qkt_chunk_units(0, 1)],
                [vm[12]], [vm[13]], [vm[14]], [],
            ])
            for j in range(2, NCH):
                lazy.append(qkt_chunk_units(0, j))
            for mi in (1, 4):
                for j in range(NCH):
                    lazy.append(qkt_chunk_units(mi, j))

            for pr in range(3):
                if pr == 1:
                    for mi in (2, 5):
                        for j in range(NCH):
                            lazy.append(qkt_chunk_units(mi, j))
                for c in range(NCH):
                    # proj for chunk c-1 becomes available only now: its
                    # outt dep (the normalize muls) needs a full chunk of
                    # latency headroom, else a proj matmul waiting on outt
                    # head-of-line-blocks the in-order PE queue.
                    if pr == 2 and c > 0:
                        for ti in range(4 * (c - 1), 4 * c):
                            lazy_out.append(proj_units(ti))
                    pair_chunk(pr, c)
            # close the final chunk inline (no next chunk to defer into)
            prev_close[0](True)
            while urgent_sched or lazy or lazy_out:
                if urgent_sched:
                    for g in urgent_sched.pop(0):
                        run_group(g)
                elif lazy:
                    run_group(lazy.pop(0))
                else:
                    run_group(lazy_out.pop(0))
            for ti in range(4 * (NCH - 1), 4 * NCH):
                run_group(proj_units(ti))

    nc.compile()
    return nc


def _prep_inputs(x, qkv_w, qkv_b):
    bf = ml_dtypes.bfloat16
    in_maps = []
    for c in range(8):
        b, hs = c // 2, (c % 2) * HL
        xt = np.zeros((KS * P, NT), dtype=bf)
        xt[0:C, :] = x[b].T.astype(bf)
        xt[C, :] = 1.0
        wq = np.zeros((KS * P, 3 * QK), dtype=bf)
        for s in range(3):  # q, k, v sections
            cols = qkv_w[:, s * C + hs * D: s * C + (hs + HL) * D]
            wq[0:C, s * QK:(s + 1) * QK] = cols.astype(bf)
        wq[C, 0:QK] = qkv_b[hs * D:(hs + HL) * D].astype(bf)
        wq[C, QK:2 * QK] = qkv_b[C + hs * D: C + (hs + HL) * D].astype(bf)
        qk_bias = np.concatenate([
            qkv_b[hs * D:(hs + HL) * D], qkv_b[C + hs * D: C + (hs + HL) * D]
        ]).astype(np.float32)
        in_maps.append({"xt": xt, "wq": wq,
                        "bias_qk": np.ascontiguousarray(qk_bias.reshape(6, P).T)})
    return in_maps


def kernel(x, qkv_w, qkv_b, proj_w, proj_b):
    from concourse.bass_utils import run_bass_kernel_spmd

    x = np.asarray(x, dtype=np.float32)
    qkv_w = np.asarray(qkv_w, dtype=np.float32)
    qkv_b = np.asarray(qkv_b, dtype=np.float32)
    proj_w = np.asarray(proj_w, dtype=np.float32)
    proj_b = np.asarray(proj_b, dtype=np.float32)

    if "nc" not in _cache:
        _cache["nc"] = _build()
    nc = _cache["nc"]

    bf = ml_dtypes.bfloat16
    in_maps = _prep_inputs(x, qkv_w, qkv_b)
    for c in range(8):
        hs = (c % 2) * HL
        in_maps[c]["wp"] = proj_w[hs * D:(hs + HL) * D, :].astype(bf)

    res = run_bass_kernel_spmd(nc, in_maps, core_ids=list(range(8)))
    parts = [res.results[c]["out"].astype(np.float32) for c in range(8)]

    # v-bias contribution (exact, f32) + proj bias, added once per batch
    const_row = qkv_b[2 * C:] @ proj_w + proj_b
    out = np.empty((B, N, C), dtype=np.float32)
    for b in range(B):
        out[b] = parts[2 * b] + parts[2 * b + 1] + const_row
    return out


# revision 45
# speedup vs baseline: 1.0198x; 1.0088x over previous
"""Multi-head self-attention (B=4, N=2048, C=768, H=12, D=64) on 8 TRN2 NeuronCores.

Sharding: (batch, head-group) — core c handles batch c//2, heads (c%2)*6..(c%2)*6+5.
Each core computes its 6 heads' attention plus the partial output projection;
the host sums the two partials per batch and adds the bias terms.

Per-core dataflow (all transpose-free):
  inputs (host-prepped, bf16):
    xt  [896, 2048]  x[b].T padded: rows 0..767 = x.T, row 768 = ones, rest 0
    wq  [896, 1152]  cols [q(384) | k(384) | v(384)] for this core's heads;
                     row 768 = [q bias | k bias | 0]
    wp  [384, 768]   proj_w rows for this core's heads
  phase 1 (ridden as fillers inside the attention loop):
    QT,KT [384, 2048] = wq[:, :768].T @ xt   (bias added on psum drain)
    V_aug [2048, (6, 128)] = xt.T @ wq[:, 768:]  (+ ones blocks)
  attention, per head PAIR (hA even on partitions 0:64, hB odd on 64:128),
  per q-chunk of 512, per k-tile m of 128:
    S^T pair tile [128, 1024] = two ROW-TILED concurrent K=64 matmuls
      (A: rows 0-63 -> psum cols 0:512, B: rows 64-127 -> cols 512:1024)
    exp split by half: A half on the scalar engine (table exp, scale fused),
      B half on the vector engine (deg-3 poly^4) — the two engines stream
      concurrently so exp is never the pipeline pacer
    mm3 per head: psum[128,512] += vv_m^T @ e_half (ones cols give denominator)
    PE queue order: mm2(m), mm3(m-1) — so PE never waits on the exp of m
  drain + normalize per (pair, chunk): reciprocal of the denominator row via
  DRAM round-trip partition-broadcast (heads A/B on separate DMA queues),
  multiply into OUT^T (head A on DVE, head B on Pool)
  proj: partial = OUT^T.T @ wp -> DRAM (fillers, per 128-token tile)
Host: out[b] = part[2b] + part[2b+1] + (qkv_b_v @ proj_w + proj_b)

Engine budget per m-iteration (~1.25us target, PE-bound):
  PE:   mm2 pair (~430ns bus) + mm3 pair (~430) + ~1.5 filler units (~320)
  ACT:  exp A-half (~700) + psum drains (vv casts, proj stage, un B)
  DVE:  exp B-half (~730) + qkt drains + un A + recip + mul A
  Pool: memsets, mul B, head-B normalize DMA chain (queue)
  Sync: input loads, head-A normalize chain, out DMAs
"""

import numpy as np
import ml_dtypes

B, N, C = 4, 2048, 768
H, D = 12, 64
SCALE = D ** -0.5
HL = 6            # heads per core
QK = HL * D       # 384, width of q (= k = v) section per core
KS = 7            # K subtiles (896 = 7*128 rows incl ones/bias row + pad)
KC = KS - 1       # 6 contraction subtiles actually used by matmuls
P = 128
NT = N            # tokens
SC = 512          # q-chunk width / phase-1 chunk width
NCH = NT // SC    # 4 q-chunks
MT = NT // P      # 16 k-tiles

_cache = {}

# DVE polynomial exp: exp(SCALE*x) ~= p(x)^4 with p a deg-3 Horner whose
# coefficients fold in SCALE/4 (minimax on the logit range |SCALE*x|<=2.1,
# max rel err ~2e-3 + bf16 out rounding). Runs on the Vector engine for the
# B-head half of every S tile so the exp stream is split across ACT+DVE.
_EXP_S = SCALE / 4
EXP_C0 = 0.16341808 * _EXP_S ** 3   # s0
EXP_C1 = 0.50925128 * _EXP_S ** 2   # s1
EXP_C2 = 1.00096638 * _EXP_S        # imm2


def _register_exp_op():
    from concourse import dve_ops as D
    from concourse.dve_spec import Spec, Src0, C0, C1, C2, One, sq, lower
    from concourse.dve_uop import DveOpSpec

    for op in D.OPS:
        if op.name == "EXP_POLY_ANT":
            return op
    body = sq(sq(((Src0 * C0 + C1) * Src0 + C2) * Src0 + One))
    spec = Spec(body=body)
    row = D._CUSTOM_DVE_ROW_BASE + len(D.OPS)
    shas = {}
    for ver in ("v3", "v4"):
        tmp = DveOpSpec(name="EXP_POLY_ANT", opcode=row,
                        uops=lower(spec, ver=ver), rd1_en=False)
        shas[ver] = tmp.sha(ver)
    op = D.DveOp("EXP_POLY_ANT", spec, subdim=False, uops_sha=shas)
    D.OPS.append(op)
    D._SUB_OPCODE_FOR_NAME[op.name] = row
    D.CUSTOM_DVE_SPECS[op.name] = spec
    return op


def _build():
    import concourse.bass as bass
    import concourse.mybir as mybir
    import concourse.tile as tile
    from concourse import bacc

    f32 = mybir.dt.float32
    bf16 = mybir.dt.bfloat16
    f8 = mybir.dt.float8e4

    nc = bacc.Bacc(None, target_bir_lowering=False)
    xt_d = nc.declare_dram_parameter("xt", [KS * P, NT], bf16, isOutput=False)
    wq_d = nc.declare_dram_parameter("wq", [KS * P, 3 * QK], bf16, isOutput=False)
    wp_d = nc.declare_dram_parameter("wp", [QK, C], bf16, isOutput=False)
    bias_d = nc.declare_dram_parameter("bias_qk", [P, 2 * QK // P], f32, isOutput=False)
    out_d = nc.declare_dram_parameter("out", [NT, C], f32, isOutput=True)

    xt_r = xt_d.rearrange("(o p) n -> p o n", p=P)
    wq_r = wq_d.rearrange("(o p) n -> p o n", p=P)
    wp_r = wp_d.rearrange("(o p) n -> p o n", p=P)

    exp_op = _register_exp_op()

    with tile.TileContext(nc) as tc:
        with (
            tc.tile_pool(name="persist", bufs=1) as persist,
            tc.tile_pool(name="e_pool", bufs=12) as e_pool,
            tc.tile_pool(name="un_pool", bufs=4) as un_pool,
            tc.tile_pool(name="rec_pool", bufs=4) as rec_pool,
            tc.tile_pool(name="bc_pool", bufs=4) as bc_pool,
            tc.tile_pool(name="stage_pool", bufs=3) as stage_pool,
            tc.tile_pool(name="dr", bufs=4, space="DRAM") as dr_pool,
            tc.tile_pool(name="psS", bufs=2, space="PSUM") as psS,
            tc.tile_pool(name="psO", bufs=1, space="PSUM") as psO,
            tc.tile_pool(name="psF", bufs=2, space="PSUM") as psF,
        ):
            # input SBUF tiles are split per DMA so every consumer's
            # semaphore wait targets exactly the transfers it needs (one
            # merged tile makes the first matmul wait for ALL writes to it)
            xt0 = [persist.tile([P, SC], bf16, name=f"xt0_{o}")
                   for o in range(KC)]                   # chunk 0, per subtile
            xtc = [None] + [persist.tile([P, KC, SC], bf16, name=f"xtc{j}")
                            for j in range(1, NCH)]      # chunks 1-3
            wqk = [persist.tile([P, QK], bf16, name=f"wqk{o}")
                   for o in range(KC)]                   # k section, per subtile
            wqq = persist.tile([P, KC, QK], bf16)        # q section
            wqv = persist.tile([P, KC, QK], bf16)        # v section
            wp = persist.tile([P, QK // P, C], bf16)
            bias_qk = persist.tile([P, 2 * QK // P], f32)
            # per-head Q^T/K^T: head h's 64 dims live at partitions
            # (h%2)*64..+64; the other half is never touched (row-tiled
            # matmuls only read the live half).
            qt = [persist.tile([P, NT], bf16, name=f"qt{h}") for h in range(HL)]
            kt = [persist.tile([P, NT], bf16, name=f"kt{h}") for h in range(HL)]
            # V_aug in fp8e4 (e and v both fp8: mm3 runs in DoubleRow perf
            # mode — two k-tiles contracted per pass at 0.5 cycles/row, ~4x
            # bf16 mm3 throughput; adds ~1e-2 rel err, gate is 2e-2).
            # Layout [P, k-tile-pair, pair-member, head, col].
            vv = persist.tile([P, MT // 2, 2, HL, P], f8)
            outt = [persist.tile([P, NT], bf16, name=f"outt{o}")
                    for o in range(QK // P)]            # normalized out^T

            eng = [nc.sync, nc.gpsimd]   # normalize DMA queues: head A / head B

            def xt_ap(o, lo, hi):
                # view of x^T subtile o, token range [lo, hi) (chunk-aligned)
                j = lo // SC
                if j == 0:
                    return xt0[o][:, lo:hi]
                return xtc[j][:, o, lo - j * SC:hi - j * SC]

            # critical-path loads interleaved across both DGE queues in
            # first-need order: kt needs (wqk[o], xt0[o]) pairs, then wqq,
            # then wqv + later xt chunks. Queue transfers serialize per
            # queue, so issue order = priority. The scalar (ACT) queue
            # carries nothing — it must stay free for the exp stream.
            nc.gpsimd.dma_start(bias_qk[:, :], bias_d[:, :])
            for o in range(KC):
                qa = nc.sync if o % 2 == 0 else nc.gpsimd
                qb = nc.gpsimd if o % 2 == 0 else nc.sync
                qa.dma_start(wqk[o][:, :], wq_r[:, o, QK:2 * QK])
                qb.dma_start(xt0[o][:, :], xt_r[:, o, 0:SC])
            nc.sync.dma_start(wqq[:, :, :], wq_r[:, 0:KC, 0:QK])
            nc.gpsimd.dma_start(wqv[:, :, :], wq_r[:, 0:KC, 2 * QK:3 * QK])
            for j in range(1, NCH):
                nc.sync.dma_start(
                    xtc[j][:, :, :], xt_r[:, 0:KC, j * SC:(j + 1) * SC])

            nc.gpsimd.dma_start(wp[:, :, :], wp_r[:, :, :])
            # V_aug col layout (M=128 so psum writes start at partition 0):
            #   even heads: [v(64) | ones(32) | zeros(32)]  -> den row 64
            #   odd  heads: [zeros(32) | ones(32) | v(64)]  -> den row 32
            # memsets on the gpsimd engine AFTER its input DMAs (~12us of
            # memset on the DVE queue would delay the first poly exp by
            # ~5us; head h's regions are only needed by pair h//2's first
            # mm3, long after the gpsimd queue drains)
            for h in range(HL):
                nc.gpsimd.memset(
                    vv[:, :, :, h, 32:96] if h % 2 else vv[:, :, :, h, 64:96], 1.0)
                nc.gpsimd.memset(
                    vv[:, :, :, h, 0:32] if h % 2 else vv[:, :, :, h, 96:128], 0.0)

            # fillers: independent PE work injected into the exp-wait slots.
            # urgent_sched is a strict per-iteration schedule (one SLOT per
            # run_filler call, each slot = a list of whole unit groups) built
            # in earliest-deadline order so every group is emitted before its
            # consuming matmul; lazy groups smooth into later slots.
            #
            # CORRECTNESS: the tile framework derives dataflow from EMISSION
            # order, so any unit writing a tensor a main-loop matmul reads
            # must be emitted first.
            # GROUP-ATOMICITY: a partially-run psum-accumulation group must
            # finish before a group of another key starts, else the psF
            # 2-buffer rotation hands a third group the bank a partial group
            # is accumulating in and its start=True wipes the partial sums.
            urgent_sched = []
            lazy = []       # qkt chunk groups: no outt dep, pop any time
            lazy_out = []   # proj groups: outt dep, pop only late in a chunk

            def run_filler(m):
                if urgent_sched:
                    for g in urgent_sched.pop(0):
                        run_group(g)
                elif lazy and m % 2 == 0:
                    run_group(lazy.pop(0))
                elif lazy_out and m >= 12:
                    # proj groups read outt written by the PREVIOUS chunk's
                    # normalize mul, which lands ~m==11 of this chunk (the
                    # deferred finish chain carries multi-us completion-
                    # semaphore latency per DMA hop) — an earlier pop would
                    # head-of-line-block the in-order PE queue
                    run_group(lazy_out.pop(0))

            def run_group(units):
                for u in units:
                    u()

            def qkt_chunk_units(mi, j):
                # one 512-col chunk of rows mi*128.. of [Q^T; K^T] (mi<3 -> Q)
                dst = qt if mi < 3 else kt
                ti = mi % 3
                cell = {}

                def unit(o, mi=mi, j=j):
                    if o == 0:
                        cell["ps"] = psF.tile([P, SC], f32, tag="fps", name="ps_f")
                    if mi < 3:
                        lhsT = wqq[:, o, mi * P:(mi + 1) * P]
                    else:
                        lhsT = wqk[o][:, (mi - 3) * P:(mi - 2) * P]
                    nc.tensor.matmul(
                        cell["ps"][:, :SC],
                        lhsT=lhsT,
                        rhs=xt_ap(o, j * SC, (j + 1) * SC),
                        start=(o == 0),
                        stop=(o == KC - 1),
                    )
                    if o == KC - 1:
                        # drain split across ACT/DVE so neither queue eats
                        # the whole ~900ns while the exp stream runs
                        sc = slice(j * SC, (j + 1) * SC)
                        nc.scalar.add(
                            out=dst[2 * ti][0:64, sc], in_=cell["ps"][0:64, :SC],
                            add=bias_qk[0:64, mi:mi + 1])
                        nc.vector.tensor_scalar_add(
                            out=dst[2 * ti + 1][64:P, sc], in0=cell["ps"][64:P, :SC],
                            scalar1=bias_qk[64:P, mi:mi + 1])
                return [lambda o=o: unit(o) for o in range(KC)]

            def v_mtile_units(ti):
                cell = {}

                def unit(o, ti=ti):
                    # subtile KS-1 is exactly zero for V (bias row of the
                    # v-section is zero and the pad rows are zero) - skip it
                    if o == 0:
                        cell["ps"] = psF.tile([P, SC], f32, tag="fps", name="ps_f")
                    nc.tensor.matmul(
                        cell["ps"][:, :QK],
                        lhsT=xt_ap(o, ti * P, (ti + 1) * P),
                        rhs=wqv[:, o, :],
                        start=(o == 0),
                        stop=(o == KC - 1),
                    )
                    if o == KC - 1:
                        psv = cell["ps"][:, :QK].rearrange("p (h d) -> p h d", h=HL)
                        # psum->fp8 casts on ACT (DVE carries the poly exp)
                        nc.scalar.copy(out=vv[:, ti // 2, ti % 2, 0:HL:2, 0:64],
                                       in_=psv[:, 0:HL:2, :])
                        nc.scalar.copy(out=vv[:, ti // 2, ti % 2, 1:HL:2, 64:128],
                                       in_=psv[:, 1:HL:2, :])
                return [lambda o=o: unit(o) for o in range(KC)]

            def proj_units(ti):
                cell = {}

                def unit(pi, o, w0, wn, ti=ti):
                    if pi == 0 and o == 0:
                        cell["stage"] = stage_pool.tile([P, C], f32, name="stage")
                    if o == 0:
                        cell["ps"] = psF.tile([P, SC], f32, tag="fps", name="ps_pj")
                    nc.tensor.matmul(
                        cell["ps"][:, :wn],
                        lhsT=outt[o][:, ti * P:(ti + 1) * P],
                        rhs=wp[:, o, w0:w0 + wn],
                        start=(o == 0),
                        stop=(o == QK // P - 1),
                    )
                    if o == QK // P - 1:
                        nc.scalar.copy(out=cell["stage"][:, w0:w0 + wn],
                                       in_=cell["ps"][:, :wn])
                        if pi == 1:
                            nc.sync.dma_start(out_d[ti * P:(ti + 1) * P, :],
                                              cell["stage"][:, :])
                return [lambda pi=pi, o=o, w0=w0, wn=wn: unit(pi, o, w0, wn)
                        for pi, (w0, wn) in enumerate([(0, 512), (512, 256)])
                        for o in range(QK // P)]

            def normalize_pair(hA, ps_oA, hB, ps_oB, cs, tail=False):
                # phase 0 (emitted NOW): drain both psums on two engines in
                # parallel (the next chunk's mm3 start only waits on these)
                # and launch the denominator-row partition-scatter DMAs
                # ([1,cw] -> [128,cw/128]; a 1-partition DVE reciprocal
                # costs ~4us, the 128-lane one ~0.2us).
                # phase 1/2 (returned as closures, emitted a few iterations
                # into the NEXT chunk): 128-lane reciprocal + DRAM
                # round-trip partition-broadcast, then the normalize muls.
                # Deferring them means their inputs (DMA round trips with
                # ~1.3us completion-semaphore latency) have already landed,
                # so they never head-of-line-block the DVE/Pool queues.
                # Head A rides sync-queue, head B gpsimd.
                cw = SC
                uns, dnps, rcps, bcs = {}, {}, {}, {}
                for idx, (h, ps_o) in enumerate(((hA, ps_oA), (hB, ps_oB))):
                    un = un_pool.tile([P, cw], f32, tag="un", name="un")
                    if idx == 0:
                        nc.vector.tensor_copy(out=un[:, :], in_=ps_o[:, :])
                    else:
                        nc.scalar.copy(out=un[:, :], in_=ps_o[:, :])
                    uns[h] = un
                for idx, h in enumerate((hA, hB)):
                    dlane = 64 if h % 2 == 0 else 32
                    dnp = rec_pool.tile([P, cw // P], f32, name="dnp", tag="dnp")
                    eng[idx].dma_start(dnp[:, :], uns[h][dlane:dlane + 1, 0:cw])
                    dnps[h] = dnp

                def finish1():
                    for idx, h in enumerate((hA, hB)):
                        rcp = rec_pool.tile([P, cw // P], f32, name="rcp", tag="rcp")
                        nc.vector.reciprocal(rcp[:, :], dnps[h][:, :])
                        rcps[h] = rcp
                    for idx, h in enumerate((hA, hB)):
                        de = eng[idx]
                        rd = dr_pool.tile([1, cw], f32, name="rd", tag="rd")
                        de.dma_start(rd[0].rearrange("(p f) -> p f", p=P),
                                     rcps[h][:, :])
                        bc = bc_pool.tile([P, cw], f32, name="bc", tag="bc")
                        de.dma_start(
                            bc[:, :],
                            bass.AP(tensor=rd.tensor, offset=rd.offset,
                                    ap=[[0, P]] + list(rd.ap)),
                        )
                        bcs[h] = bc

                def finish2():
                    # both muls on Pool: it idles otherwise, and a wait at
                    # its queue head blocks nothing (DVE must stay clear
                    # for the poly stream)
                    for idx, h in enumerate((hA, hB)):
                        t, po = h // 2, (h % 2) * 64
                        mul = (nc.vector.tensor_mul if tail
                               else nc.gpsimd.tensor_mul)
                        mul(outt[t][po:po + 64, cs], uns[h][po:po + 64, 0:cw],
                            bcs[h][po:po + 64, :])

                if tail:
                    finish1()
                    finish2()
                    return None
                return finish1, finish2

            pending = [None]      # deferred normalize finish of chunk c-2
            prev_close = [None]   # emits mm3(15) + normalize ph0 of chunk c-1

            def pair_chunk(pr, c):
                # one q-chunk of 512 for head pair (2*pr, 2*pr+1): row-tiled
                # S^T pair per k-tile, exp split A-half on ACT / B-half on
                # DVE (poly), per-head mm3 with the PE-order trick (mm3 of
                # m-1 queued after mm2 of m). The LAST mm3 + normalize
                # drains of a chunk are emitted at m==0 of the NEXT chunk
                # (cross-chunk software pipeline) so the exp stream never
                # drains at a chunk boundary.
                hA, hB = 2 * pr, 2 * pr + 1
                qs = slice(c * SC, (c + 1) * SC)
                ps_oA = psO.tile([P, SC], f32, tag="oa", name="ps_oa")
                ps_oB = psO.tile([P, SC], f32, tag="ob", name="ps_ob")
                es = [None] * (MT // 2)
                for m in range(MT):
                    ps = psS.tile([P, 2 * SC], f32, tag="ps", name="ps_s")
                    ms = slice(m * P, (m + 1) * P)
                    nc.tensor.matmul(
                        ps[:, 0:SC], lhsT=kt[hA][0:64, ms], rhs=qt[hA][0:64, qs],
                        start=True, stop=True,
                    )
                    nc.tensor.matmul(
                        ps[:, SC:2 * SC], lhsT=kt[hB][64:P, ms], rhs=qt[hB][64:P, qs],
                        start=True, stop=True,
                    )
                    # e pair tile [P, pair-member, head-half * SC] in fp8:
                    # member m%2 holds exp of k-tile m, matching vv's pair
                    # layout for the DoubleRow mm3
                    if m % 2 == 0:
                        es[m // 2] = e_pool.tile([P, 2, 2 * SC], f8, name="e")
                    e = es[m // 2]
                    nc.scalar.activation(
                        e[:, m % 2, 0:SC], ps[:, 0:SC],
                        mybir.ActivationFunctionType.Exp,
                        scale=float(SCALE),
                    )
                    nc.vector._custom_dve(exp_op, out=e[:, m % 2, SC:2 * SC],
                                          in0=ps[:, SC:2 * SC],
                                          s0=EXP_C0, s1=EXP_C1, imm2=EXP_C2)
                    if m == 0 and prev_close[0] is not None:
                        prev_close[0](False)
                    if pending[0] is not None:
                        # recips at m==3: their dnp inputs (issued at m==0)
                        # have landed, so no DVE head-of-line wait. During
                        # pair 2 flush the muls right after at m==5: they
                        # camp at the idle Pool queue head until the bc
                        # semaphore (~m==8) and finish by ~m==10, so the
                        # m>=12 proj pops never stall
                        f1, f2 = (3, 5) if pr == 2 else (3, 8)
                        if m == f1:
                            pending[0][0]()
                        elif m == f2:
                            pending[0][1]()
                            pending[0] = None
                    run_filler(m)
                    # consume e pairs two iterations late: mm3(mp) sits on
                    # the in-order PE queue after mm2(2mp+3), so PE never
                    # waits for the exp stream
                    if m % 2 == 1 and m >= 3:
                        mm3(hA, hB, (m - 3) // 2, ps_oA, ps_oB, es[(m - 3) // 2])

                def close(tail):
                    mm3(hA, hB, MT // 2 - 1, ps_oA, ps_oB, es[MT // 2 - 1])
                    pending[0] = normalize_pair(hA, ps_oA, hB, ps_oB, qs,
                                                tail=tail)
                prev_close[0] = close

            def mm3(hA, hB, mp, ps_oA, ps_oB, e):
                # fp8 DoubleRow: contracts k-tile pair (2mp, 2mp+1) in one
                # pass (0.5 cycles/row). lhsT [128, 2, cols], rhs [128, 2,
                # SC] — dim1 is the pair member. Even heads: [v(64)|ones(32)]
                # is all that matters - 96 stationary cols shave the
                # LDWEIGHTS.
                dr = mybir.MatmulPerfMode.DoubleRow
                nc.tensor.matmul(
                    ps_oA[0:96, :], lhsT=vv[:, mp, :, hA, 0:96],
                    rhs=e[:, :, 0:SC], perf_mode=dr,
                    start=(mp == 0), stop=(mp == MT // 2 - 1),
                )
                nc.tensor.matmul(
                    ps_oB[:, :], lhsT=vv[:, mp, :, hB, :],
                    rhs=e[:, :, SC:2 * SC], perf_mode=dr,
                    start=(mp == 0), stop=(mp == MT // 2 - 1),
                )

            # pre-work: K^T chunk 0, Q^T chunk 0 of pair 0, and V tile 0 —
            # just enough to start the exp stream. Everything else rides the
            # filler slots; urgent_sched is the hand-scheduled chunk-0 plan
            # (v_m needed by mm3(m) emitted in iteration m+1; kt-j1/2/3 by
            # mm2(4)/(8)/(12); qt(0,1) by pair-0-chunk-1's first mm2).
            run_group(qkt_chunk_units(3, 0))
            run_group(qkt_chunk_units(0, 0))
            run_group(v_mtile_units(0))
            kj = [qkt_chunk_units(3, j) for j in range(1, NCH)]
            vm = [v_mtile_units(ti) for ti in range(1, MT)]
            # qt chunk 1 rides slot 11 (double-popped with v12): computed at
            # the last slot it would finish just-in-time and stall the
            # pc0->pc1 transition ~4us waiting on its psum drains
            urgent_sched.extend([
                [vm[0]], [vm[1], kj[0]], [vm[2]], [vm[3]],
                [vm[4], kj[1]], [vm[5]], [vm[6]], [vm[7]],
                [vm[8]], [vm[9], kj[2]], [vm[10]],
                [vm[11], qkt_chunk_units(0, 1)],
                [vm[12]], [vm[13]], [vm[14]], [],
            ])
            for j in range(2, NCH):
                lazy.append(qkt_chunk_units(0, j))
            for mi in (1, 4):
                for j in range(NCH):
                    lazy.append(qkt_chunk_units(mi, j))

            for pr in range(3):
                if pr == 1:
                    for mi in (2, 5):
                        for j in range(NCH):
                            lazy.append(qkt_chunk_units(mi, j))
                for c in range(NCH):
                    # proj for chunk c-1 becomes available only now: its
                    # outt dep (the normalize muls) needs a full chunk of
                    # latency headroom, else a proj matmul waiting on outt
                    # head-of-line-blocks the in-order PE queue.
                    if pr == 2 and c > 0:
                        for ti in range(4 * (c - 1), 4 * c):
                            lazy_out.append(proj_units(ti))
                    pair_chunk(pr, c)
            # close the final chunk inline (no next chunk to defer into)
            prev_close[0](True)
            while urgent_sched or lazy or lazy_out:
                if urgent_sched:
                    for g in urgent_sched.pop(0):
                        run_group(g)
                elif lazy:
                    run_group(lazy.pop(0))
                else:
                    run_group(lazy_out.pop(0))
            for ti in range(4 * (NCH - 1), 4 * NCH):
                run_group(proj_units(ti))

    nc.compile()
    return nc


def _prep_inputs(x, qkv_w, qkv_b):
    bf = ml_dtypes.bfloat16
    in_maps = []
    for c in range(8):
        b, hs = c // 2, (c % 2) * HL
        xt = np.zeros((KS * P, NT), dtype=bf)
        xt[0:C, :] = x[b].T.astype(bf)
        xt[C, :] = 1.0
        wq = np.zeros((KS * P, 3 * QK), dtype=bf)
        for s in range(3):  # q, k, v sections
            cols = qkv_w[:, s * C + hs * D: s * C + (hs + HL) * D]
            wq[0:C, s * QK:(s + 1) * QK] = cols.astype(bf)
        wq[C, 0:QK] = qkv_b[hs * D:(hs + HL) * D].astype(bf)
        wq[C, QK:2 * QK] = qkv_b[C + hs * D: C + (hs + HL) * D].astype(bf)
        qk_bias = np.concatenate([
            qkv_b[hs * D:(hs + HL) * D], qkv_b[C + hs * D: C + (hs + HL) * D]
        ]).astype(np.float32)
        in_maps.append({"xt": xt, "wq": wq,
                        "bias_qk": np.ascontiguousarray(qk_bias.reshape(6, P).T)})
    return in_maps


def kernel(x, qkv_w, qkv_b, proj_w, proj_b):
    from concourse.bass_utils import run_bass_kernel_spmd

    x = np.asarray(x, dtype=np.float32)
    qkv_w = np.asarray(qkv_w, dtype=np.float32)
    qkv_b = np.asarray(qkv_b, dtype=np.float32)
    proj_w = np.asarray(proj_w, dtype=np.float32)
    proj_b = np.asarray(proj_b, dtype=np.float32)

    if "nc" not in _cache:
        _cache["nc"] = _build()
    nc = _cache["nc"]

    bf = ml_dtypes.bfloat16
    in_maps = _prep_inputs(x, qkv_w, qkv_b)
    for c in range(8):
        hs = (c % 2) * HL
        in_maps[c]["wp"] = proj_w[hs * D:(hs + HL) * D, :].astype(bf)

    res = run_bass_kernel_spmd(nc, in_maps, core_ids=list(range(8)))
    parts = [res.results[c]["out"].astype(np.float32) for c in range(8)]

    # v-bias contribution (exact, f32) + proj bias, added once per batch
    const_row = qkv_b[2 * C:] @ proj_w + proj_b
    out = np.empty((B, N, C), dtype=np.float32)
    for b in range(B):
        out[b] = parts[2 * b] + parts[2 * b + 1] + const_row
    return out
